# revision 7
# baseline (speedup 1.0000x reference)
"""Trainium2 Bass kernel for nn_EnvironmentEmbedder (v15).

Sharding: pure data parallel; core i takes batch slice [128*i:128*(i+1)],
batch elements = SBUF partitions.

DMA plan (~51 DMAs total; each HWDGE ring runs FIFO with a per-DMA gap, so
few/big/spread-out transfers):
  - SP ring:   static-chunk loads  -> sd[:, :w]      (18 DMAs)
  - ACT ring:  dynamic-chunk loads -> sd[:, w:2w]    (18 DMAs, concurrent)
  - Pool ring (SWDGE): small_pack load, 3 shuffle gathers (behind the first
    env store so they don't block o-tile recycling), env stores (2 chunks
    per DMA), one merged 33-channel stage store.
  - Chunk widths taper 4,4,(8x14),4,4 so the first add starts early and the
    end-of-stream serial drain (load->add->mul->store) is short.

Compute:
  - shuffle out_j = x_{(j-rot)%6} is an indirect gather with one
    per-partition index over a host-doubled buffer [x x];
  - env: DVE add f32+f32 -> bf16 (rounding the exact f32 sum is a
    multiplicative error, safe at cancellation), then all-bf16 in-place mul
    vs a pre-replicated obs tile (2x DVE mode);
  - 33 small channels staged in one tile; every emitted item is a SINGLE
    DVE/ACT op so the per-chunk budget caps DVE hiccups (sd pool is only
    2 deep).

Output layout per core: [128, 161*625] bf16, channel-major:
  ch 0..127   (s+d)*obs          ch 138..139 leader/follower*obs
  ch 128..130 obst/ocur/obs *obs ch 140..145 shuf(atgt)*(0.5*obs)
  ch 131..136 shuf(vis)*(0.5*obs)ch 146..151 shuf(ptgt)*obs
  ch 137      sum(vis)*obs       ch 152..153 0.5*sum(atgt)/sum(ptgt) *obs
                                 ch 154..160 ones, one_hot(rot)
"""

import sys

sys.path.insert(0, "/opt/trn_rl_repo")

from contextlib import ExitStack

import ml_dtypes
import numpy as np

import concourse.bass as bass
import concourse.tile as tile
from concourse import bacc, mybir
from concourse.bass import IndirectOffsetOnAxis
from concourse.bass_utils import run_bass_kernel_spmd

F32 = mybir.dt.float32
BF16 = mybir.dt.bfloat16
I32 = mybir.dt.int32
ALU = mybir.AluOpType
NP_BF16 = ml_dtypes.bfloat16

B = 1024
N_CORES = 8
BS = B // N_CORES
EMB = 128
HW = 625
NROT = 6
NCH = EMB + 33  # 161

CHUNKS = [4, 4] + [8] * 14 + [4, 4]   # channel widths, sum = 128
MAXW = 8 * HW                          # widest chunk in columns

SMALL_NAMES = ["obs", "obstacle", "ocur", "leader", "follower"]
SMALL_W = 5 * HW  # +1 bf16 rot column rides at the end


def build_body(nc, tc, ctx, t_in, t_out):
    pool = ctx.enter_context(tc.tile_pool(name="resident", bufs=1))
    sd_pool = ctx.enter_context(tc.tile_pool(name="sd", bufs=2))
    o_pool = ctx.enter_context(tc.tile_pool(name="o", bufs=2))
    st_pool = ctx.enter_context(tc.tile_pool(name="stage", bufs=1))

    # ---- small pack on the Pool ring; rot rides as a bf16 column ----
    small_t = pool.tile([BS, SMALL_W + 1], BF16, tag="small")
    nc.gpsimd.dma_start(small_t[:], t_in["small_pack"][:, :])
    cols = {nm: small_t[:, i * HW:(i + 1) * HW] for i, nm in enumerate(SMALL_NAMES)}
    obs_t = cols["obs"]
    rot_t = pool.tile([BS, 1], I32, tag="rot")
    nc.vector.tensor_copy(rot_t[:], small_t[:, SMALL_W:SMALL_W + 1])

    def bc(ap, n):
        return ap.rearrange("p (o x) -> p o x", o=1).to_broadcast([BS, n, HW])

    obs8 = pool.tile([BS, MAXW], BF16, tag="obs8")
    obs6 = obs8[:, :NROT * HW]

    # ---- gather indices: idx[p] = p*12 + (6 - rot_p) % 6 ----
    idx = pool.tile([BS, 1], I32, tag="idx")
    nc.gpsimd.iota(idx[:], [[0, 1]], base=NROT, channel_multiplier=2 * NROT)
    nc.vector.tensor_tensor(idx[:], idx[:], rot_t[:], op=ALU.subtract)
    m0 = pool.tile([BS, 1], I32, tag="m0")
    nc.vector.tensor_scalar(m0[:], rot_t[:], 0, None, op0=ALU.is_equal)
    nc.vector.tensor_scalar_mul(m0[:], m0[:], NROT)
    nc.vector.tensor_tensor(idx[:], idx[:], m0[:], op=ALU.subtract)
    # obs replica by doubling: plain step-1 bf16 copies run in 4x DVE mode
    nc.vector.tensor_copy(obs8[:, :HW], obs_t)
    nc.vector.tensor_copy(obs8[:, HW:2 * HW], obs8[:, :HW])
    nc.vector.tensor_copy(obs8[:, 2 * HW:4 * HW], obs8[:, :2 * HW])

    # ---- compass masks (created lazily in the small-op stream) ----
    R = []
    for r in range(NROT):
        rt = pool.tile([BS, 1], F32, tag=f"R{r}", name=f"R{r}")
        R.append(rt)

    scratch = pool.tile([BS, HW], F32, tag="scratch")

    sh = {}
    for nm in ("vis", "atgt", "ptgt"):
        sh[nm] = pool.tile([BS, NROT * HW], BF16, tag=f"sh_{nm}",
                           name=f"sh_{nm}")

    def emit_gathers():
        for nm in ("vis", "atgt", "ptgt"):
            nc.gpsimd.indirect_dma_start(
                sh[nm][:], None, t_in[f"{nm}2"][:, :],
                IndirectOffsetOnAxis(ap=idx[:], axis=0))

    stage = st_pool.tile([BS, 33 * HW], BF16, tag="stage")  # ch128..160
    stA = stage[:, :12 * HW]
    stB = stage[:, 12 * HW:26 * HW]
    stC = stage[:, 26 * HW:]

    def chsum_ops(dst, src6, scale, eng=None):
        # each item is (dve_cost_us, op) — one instruction per item. The add
        # chain can ride GpSimd (validated on HW in the v8 round) for the
        # late chsums, where the Pool engine is idle and DVE is the pacer;
        # the final mul stays on DVE (f32 scratch x bf16 obs).
        eng = eng or nc.vector
        cost = 0.0 if eng is nc.gpsimd else 0.9
        yield cost, lambda: eng.tensor_tensor(
            scratch[:], src6[:, 0:HW], src6[:, HW:2 * HW], op=ALU.add)
        for k in range(2, NROT):
            yield cost, lambda k=k: eng.tensor_tensor(
                scratch[:], scratch[:], src6[:, k * HW:(k + 1) * HW],
                op=ALU.add)
        if scale != 1.0:
            yield cost / 3, lambda: eng.tensor_scalar_mul(
                scratch[:], scratch[:], scale)
        yield 0.9, lambda: nc.vector.tensor_mul(dst, scratch[:], obs6[:, :HW])

    def emit_small_ops():
        # gather-independent items first so the in-order DVE stream never
        # parks head-of-line on a not-yet-landed gather tile
        yield 0.5, lambda: nc.vector.tensor_mul(stA[:, 0:HW],
                                                cols["obstacle"], obs_t)
        yield 0.5, lambda: nc.vector.tensor_mul(stA[:, HW:2 * HW],
                                                cols["ocur"], obs_t)
        yield 0.5, lambda: nc.vector.tensor_mul(stA[:, 2 * HW:3 * HW],
                                                obs_t, obs_t)
        yield 0.5, lambda: nc.vector.tensor_mul(stA[:, 10 * HW:11 * HW],
                                                cols["leader"], obs_t)
        yield 0.5, lambda: nc.vector.tensor_mul(stA[:, 11 * HW:12 * HW],
                                                cols["follower"], obs_t)
        yield 0.5, lambda: nc.vector.memset(stC[:, 0:HW], 1.0)
        for r in range(NROT):
            yield 0.1, lambda r=r: nc.vector.tensor_scalar(
                R[r][:], rot_t[:], r, None, op0=ALU.is_equal)
        for r in range(NROT):
            yield 0.0, lambda r=r: nc.scalar.activation(
                stC[:, (1 + r) * HW:(2 + r) * HW], obs_t,
                mybir.ActivationFunctionType.Identity, bias=R[r][:], scale=0.0)
        # sh-dependent work (gathers kicked at chunk 0, landed by now)
        yield 2.1, lambda: nc.vector.tensor_mul(stA[:, 3 * HW:9 * HW],
                                                sh["vis"][:], obs6[:])
        yield 1.1, lambda: nc.vector.tensor_scalar_mul(
            stA[:, 3 * HW:9 * HW], stA[:, 3 * HW:9 * HW], 0.5)
        yield from chsum_ops(stA[:, 9 * HW:10 * HW], sh["vis"], 1.0)
        yield 0.0, lambda: nc.gpsimd.dma_start(t_out[:, 128 * HW:140 * HW],
                                               stA[:])
        yield 2.1, lambda: nc.vector.tensor_mul(stB[:, 0:6 * HW],
                                                sh["atgt"][:], obs6[:])
        yield 1.1, lambda: nc.vector.tensor_scalar_mul(
            stB[:, 0:6 * HW], stB[:, 0:6 * HW], 0.5)
        yield 2.1, lambda: nc.vector.tensor_mul(stB[:, 6 * HW:12 * HW],
                                                sh["ptgt"][:], obs6[:])
        yield from chsum_ops(stB[:, 12 * HW:13 * HW], sh["atgt"], 0.5,
                             eng=nc.gpsimd)
        yield from chsum_ops(stB[:, 13 * HW:14 * HW], sh["ptgt"], 1.0,
                             eng=nc.gpsimd)

    small_iter = emit_small_ops()
    small_done = False

    def emit_small(budget_us):
        # budget is DVE-microseconds per chunk slot; ACT/store items are free
        nonlocal small_done
        while budget_us > 0 and not small_done:
            item = next(small_iter, None)
            if item is None:
                small_done = True
                return
            cost, op = item
            op()
            budget_us -= max(cost, 0.05)

    # ---- env stream ----
    o = None
    o_off = 0
    o_ch0 = 0
    off = 0  # channel offset
    for i, w in enumerate(CHUNKS):
        wcols = w * HW
        sd = sd_pool.tile([BS, 2 * MAXW], F32, tag="sd")
        nc.sync.dma_start(sd[:, :wcols],
                          t_in["env_static"][:, off * HW:(off + w) * HW])
        nc.scalar.dma_start(sd[:, wcols:2 * wcols],
                            t_in["env_dyn"][:, off * HW:(off + w) * HW])
        if i % 2 == 0:
            o = o_pool.tile([BS, 2 * MAXW], BF16, tag="o")
            o_off = 0
            o_ch0 = off
        half = o[:, o_off:o_off + wcols]
        nc.vector.tensor_tensor(half, sd[:, :wcols], sd[:, wcols:2 * wcols],
                                op=ALU.add)
        nc.vector.tensor_mul(half, half, obs8[:, :wcols])
        o_off += wcols
        if i % 2 == 1:
            ring = nc.scalar if i == len(CHUNKS) - 1 else nc.gpsimd
            ring.dma_start(t_out[:, o_ch0 * HW:(off + w) * HW], o[:, :o_off])
        if i == 0:
            emit_gathers()
        if i == 1:
            nc.vector.tensor_copy(obs8[:, 4 * HW:], obs8[:, :4 * HW])
        if 2 <= i <= 9:
            emit_small(1.2)
        elif i >= 10:
            emit_small(3.5)
        off += w
    emit_small(100.0)
    # stage-BC store drains on the SP ring, which is idle once loads end
    nc.sync.dma_start(t_out[:, 140 * HW:161 * HW], stage[:, 12 * HW:])


def build_nc():
    nc = bacc.Bacc("TRN2", target_bir_lowering=False, debug=False)
    t_in = {
        "env_static": nc.dram_tensor(
            "env_static", [BS, EMB * HW], F32, kind="ExternalInput"),
        "env_dyn": nc.dram_tensor(
            "env_dyn", [BS, EMB * HW], F32, kind="ExternalInput"),
        "small_pack": nc.dram_tensor(
            "small_pack", [BS, SMALL_W + 1], BF16, kind="ExternalInput"),
        "vis2": nc.dram_tensor(
            "vis2", [BS * 2 * NROT, HW], BF16, kind="ExternalInput"),
        "atgt2": nc.dram_tensor(
            "atgt2", [BS * 2 * NROT, HW], BF16, kind="ExternalInput"),
        "ptgt2": nc.dram_tensor(
            "ptgt2", [BS * 2 * NROT, HW], BF16, kind="ExternalInput"),
    }
    t_out = nc.dram_tensor("out", [BS, NCH * HW], BF16, kind="ExternalOutput")
    with tile.TileContext(nc) as tc, ExitStack() as ctx:
        build_body(nc, tc, ctx, t_in, t_out)
    nc.compile()
    return nc


def make_in_maps(inputs):
    arrs = {k: np.asarray(v) for k, v in inputs.items()}
    small = np.concatenate([
        arrs["observability_in_memory"].reshape(B, HW),
        arrs["obstacle_mask"].reshape(B, HW),
        arrs["observability_current"].reshape(B, HW),
        arrs["leader_location"].reshape(B, HW),
        arrs["follower_location"].reshape(B, HW),
        arrs["rotations"].reshape(B, 1).astype(np.float32),
    ], axis=1).astype(NP_BF16)

    def dbl(name):
        x = arrs[name].reshape(B, NROT * HW).astype(NP_BF16)
        return np.concatenate([x, x], axis=1)

    flat = {
        "env_static": arrs["embedded_static"].reshape(B, EMB * HW),
        "env_dyn": arrs["embedded_dynamic"].reshape(B, EMB * HW),
        "small_pack": small,
        "vis2": dbl("previous_visitations"),
        "atgt2": dbl("all_previous_targets"),
        "ptgt2": dbl("previous_target"),
    }
    maps = []
    for i in range(N_CORES):
        sl = slice(i * BS, (i + 1) * BS)
        m = {k: np.ascontiguousarray(v[sl]) for k, v in flat.items()}
        for nm in ("vis2", "atgt2", "ptgt2"):
            m[nm] = np.ascontiguousarray(m[nm].reshape(BS * 2 * NROT, HW))
        maps.append(m)
    return maps


def gather_out(results):
    return np.concatenate(
        [np.asarray(r["out"]).astype(np.float32).reshape(BS, NCH, 25, 25)
         for r in results], axis=0)


def kernel(**inputs) -> np.ndarray:
    nc = build_nc()
    in_maps = make_in_maps(inputs)
    res = run_bass_kernel_spmd(nc, in_maps, list(range(N_CORES)))
    return gather_out(res.results)


if __name__ == "__main__":
    rng = np.random.default_rng(0)
    demo = {
        "embedded_static": rng.standard_normal((B, EMB, 25, 25)).astype(np.float32),
        "embedded_dynamic": rng.standard_normal((B, EMB, 25, 25)).astype(np.float32),
        "obstacle_mask": rng.random((B, 25, 25), dtype=np.float32),
        "observability_current": rng.random((B, 25, 25), dtype=np.float32),
        "observability_in_memory": rng.random((B, 25, 25), dtype=np.float32),
        "previous_visitations": rng.random((B, NROT, 25, 25), dtype=np.float32),
        "all_previous_targets": rng.random((B, NROT, 25, 25), dtype=np.float32),
        "previous_target": rng.random((B, NROT, 25, 25), dtype=np.float32),
        "leader_location": rng.random((B, 25, 25), dtype=np.float32),
        "follower_location": rng.random((B, 25, 25), dtype=np.float32),
        "rotations": rng.integers(0, NROT, (B,), dtype=np.int32),
    }
    out = kernel(**demo)
    print("out", out.shape, out.dtype)
